# revision 2
# baseline (speedup 1.0000x reference)
"""Cross-attention + RoPE Bass/Tile kernel for TRN2 (v2 rewrite).

Per-core computation (batch element b = core id). Natural head-dim layout
(no even/odd permutation). All matmuls contract over the partition dim:

  qT  [C, NQ] tiles: qT[t] rows = head-dims of heads 2t, 2t+1 (d natural)
  kTr [C, NP] tiles: same layout, RoPE applied
  RoPE: krot = kn * cos2 + (P_swap @ kn) * sin2, where P_swap swaps
        adjacent d-pairs with sign (-odd, +even) — done as a PE matmul.
  scores per head: one matmul, 64-part contraction:
        s[128k, 512q] = kTr[g][64h', ki*128:+128].T @ qT[g][64h', qh*512:+512]
  exp in bf16 (scores/8 reaches ~16.4 > fp16 exp overflow; bf16 is safe)
  PV transposed: acc[128q, 65] += ex[128k, 128q].T @ v_aug[128k, 65]
        (v augmented with a ones column per head -> softmax denominator)
  drain: o_sb[q, c] = acc[:, 0:64] * (1/acc[:, 64])
  out-proj: PE-transpose o_sb -> oT [c, q] tiles, y = oT.T @ woT + bo
"""

import sys as _s
if "/opt/trn_rl_repo" not in _s.path:
    _s.path.insert(0, "/opt/trn_rl_repo")

import numpy as np

import concourse.bass as bass
import concourse.mybir as mybir
from concourse.bass import ts

F32 = mybir.dt.float32
F16 = mybir.dt.float16
BF16 = mybir.dt.bfloat16

B, NQ, NP, C, H = 8, 1024, 2048, 1024, 16
D = C // H  # 64
NCT = C // 128  # 8 c-tiles
SCALE = 1.0 / np.sqrt(D)
ALL_PHASES = (1, 2, 3, 4, 5)


def host_prep(x, ctx, freqs_cis, wq, bq, wk, bk, wv, bv, wo, bo):
    """Numpy-side layout prep. Returns per-core list of input dicts."""
    f32, f16 = np.float32, np.float16

    # cos2/sin2 [128, NP]: row r (within a 2-head tile) -> pair (r % 64)//2
    cosT = np.ascontiguousarray(np.asarray(freqs_cis, f32)[:, :, 0].T)  # [32, NP]
    sinT = np.ascontiguousarray(np.asarray(freqs_cis, f32)[:, :, 1].T)
    cos2 = np.tile(np.repeat(cosT, 2, axis=0), (2, 1)).astype(f16)
    sin2 = np.tile(np.repeat(sinT, 2, axis=0), (2, 1)).astype(f16)

    # P_swap (as lhsT): out[2p] = -kn[2p+1]; out[2p+1] = +kn[2p]
    pswapT = np.zeros((128, 128), dtype=f16)
    for p in range(64):
        pswapT[2 * p + 1, 2 * p] = -1.0
        pswapT[2 * p, 2 * p + 1] = 1.0
    ident = np.eye(128, dtype=f16)

    shared = {
        "wqT": np.ascontiguousarray(np.asarray(wq, f32).T.astype(f16)),
        "wkT": np.ascontiguousarray(np.asarray(wk, f32).T.astype(f16)),
        "wvT": np.ascontiguousarray(np.asarray(wv, f32).T.astype(f16)),
        "woT": np.ascontiguousarray(np.asarray(wo, f32).T.astype(f16)),
        "bq": np.asarray(bq, f32).copy(),
        "bk": np.asarray(bk, f32).copy(),
        "bv": np.asarray(bv, f32).copy(),
        "bo": np.asarray(bo, f32).copy(),
        "cos2": np.ascontiguousarray(cos2),
        "sin2": np.ascontiguousarray(sin2),
        "pswapT": pswapT,
        "ident": ident,
    }
    per_core = []
    for b in range(x.shape[0]):
        per_core.append({
            "xT": np.ascontiguousarray(np.asarray(x[b], f32).T.astype(f16)),
            "ctxT": np.ascontiguousarray(np.asarray(ctx[b], f32).T.astype(f16)),
            **shared,
        })
    return per_core


def declare_io(nc):
    d = {}
    d["xT"] = nc.dram_tensor("xT", [C, NQ], F16, kind="ExternalInput").ap()
    d["ctxT"] = nc.dram_tensor("ctxT", [C, NP], F16, kind="ExternalInput").ap()
    for w in ("wqT", "wkT", "wvT", "woT"):
        d[w] = nc.dram_tensor(w, [C, C], F16, kind="ExternalInput").ap()
    for bname in ("bq", "bk", "bv", "bo"):
        d[bname] = nc.dram_tensor(bname, [C], F32, kind="ExternalInput").ap()
    d["cos2"] = nc.dram_tensor("cos2", [128, NP], F16, kind="ExternalInput").ap()
    d["sin2"] = nc.dram_tensor("sin2", [128, NP], F16, kind="ExternalInput").ap()
    d["pswapT"] = nc.dram_tensor("pswapT", [128, 128], F16, kind="ExternalInput").ap()
    d["ident"] = nc.dram_tensor("ident", [128, 128], F16, kind="ExternalInput").ap()
    d["y"] = nc.dram_tensor("y", [NQ, C], F32, kind="ExternalOutput").ap()
    return d


def emit(ctx, tc, io, phases=ALL_PHASES):
    """Emit the kernel under an open TileContext. ctx is an ExitStack."""
    nc = tc.nc
    Exp = mybir.ActivationFunctionType.Exp

    consts = ctx.enter_context(tc.tile_pool(name="consts", bufs=1))
    weights = ctx.enter_context(tc.tile_pool(name="weights", bufs=1))
    persist = ctx.enter_context(tc.tile_pool(name="persist", bufs=1))

    # --- constants -------------------------------------------------------
    bq_sb = consts.tile([128, NCT], F32, tag="bq", name="bq")
    bk_sb = consts.tile([128, NCT], F32, tag="bk", name="bk")
    nc.sync.dma_start(out=bq_sb[:], in_=io["bq"].rearrange("(t p) -> p t", p=128))
    nc.sync.dma_start(out=bk_sb[:], in_=io["bk"].rearrange("(t p) -> p t", p=128))
    bv_row = consts.tile([1, C], F32, tag="bv_row", name="bv_row")
    bo_row = consts.tile([1, C], F32, tag="bo_row", name="bo_row")
    nc.sync.dma_start(out=bv_row[:], in_=io["bv"].unsqueeze(0))
    nc.sync.dma_start(out=bo_row[:], in_=io["bo"].unsqueeze(0))
    bv_rep = consts.tile([128, C], F32, tag="bv_rep", name="bv_rep")
    bo_rep = consts.tile([128, C], F32, tag="bo_rep", name="bo_rep")
    nc.gpsimd.partition_broadcast(bv_rep[:], bv_row[:], channels=128)
    nc.gpsimd.partition_broadcast(bo_rep[:], bo_row[:], channels=128)
    cos2 = consts.tile([128, NP], F16, tag="cos2", name="cos2")
    sin2 = consts.tile([128, NP], F16, tag="sin2", name="sin2")
    nc.sync.dma_start(out=cos2[:], in_=io["cos2"][:])
    nc.sync.dma_start(out=sin2[:], in_=io["sin2"][:])
    pswapT = consts.tile([128, 128], F16, tag="pswapT", name="pswapT")
    ident = consts.tile([128, 128], F16, tag="ident", name="ident")
    nc.sync.dma_start(out=pswapT[:], in_=io["pswapT"][:])
    nc.sync.dma_start(out=ident[:], in_=io["ident"][:])

    # --- weights (resident) ---------------------------------------------
    w_sb = {}
    for w in ("wqT", "wkT", "wvT", "woT"):
        w_sb[w] = [
            weights.tile([128, C], F16, tag=f"{w}{t}", name=f"{w}{t}")
            for t in range(NCT)
        ]
        for t in range(NCT):
            nc.sync.dma_start(out=w_sb[w][t][:], in_=io[w][ts(t, 128), :])

    # --- persistent activations -----------------------------------------
    qT = [persist.tile([128, NQ], F16, tag=f"qT{t}", name=f"qT{t}") for t in range(NCT)]
    kTr = [persist.tile([128, NP], F16, tag=f"kTr{t}", name=f"kTr{t}") for t in range(NCT)]
    v_sb = [
        persist.tile([128, H * (D + 1)], BF16, tag=f"v{kc}", name=f"v{kc}")
        for kc in range(NP // 128)
    ]
    o_sb = [persist.tile([128, C], F16, tag=f"o{sc}", name=f"o{sc}") for sc in range(NQ // 128)]

    if 1 in phases:
        _phase1_qproj(tc, nc, io, qT, w_sb["wqT"], bq_sb)
    if 2 in phases:
        _phase2_kproj_rope(tc, nc, io, kTr, w_sb["wkT"], bk_sb, pswapT, cos2, sin2)
    if 3 in phases:
        _phase3_vproj(tc, nc, io, v_sb, w_sb["wvT"], bv_rep)
    if 4 in phases:
        _phase4_attention(tc, nc, qT, kTr, v_sb, o_sb, Exp)
    if 5 in phases:
        _phase5_outproj(tc, nc, io, o_sb, w_sb["woT"], bo_rep, ident)


def _phase1_qproj(tc, nc, io, qT, wq_sb, bq_sb):
    with (
        tc.tile_pool(name="xs", bufs=1) as xs_pool,
        tc.tile_pool(name="ps1", bufs=4, space="PSUM") as ps1,
    ):
        for qs in range(NQ // 512):
            x_t = [
                xs_pool.tile([128, 512], F16, tag=f"x{ct}", name=f"x{ct}")
                for ct in range(NCT)
            ]
            for ct in range(NCT):
                nc.sync.dma_start(out=x_t[ct][:], in_=io["xT"][ts(ct, 128), ts(qs, 512)])
            for ot in range(NCT):
                ps = ps1.tile([128, 512], F32, tag="ps", name="ps")
                for ct in range(NCT):
                    nc.tensor.matmul(
                        ps[:],
                        wq_sb[ct][:, ts(ot, 128)],
                        x_t[ct][:],
                        start=(ct == 0),
                        stop=(ct == NCT - 1),
                    )
                nc.vector.tensor_scalar_add(
                    qT[ot][:, ts(qs, 512)], ps[:], bq_sb[:, ot : ot + 1]
                )


def _phase2_kproj_rope(tc, nc, io, kTr, wk_sb, bk_sb, pswapT, cos2, sin2):
    with (
        tc.tile_pool(name="cs", bufs=1) as cs_pool,
        tc.tile_pool(name="ps2", bufs=3, space="PSUM") as ps2,
        tc.tile_pool(name="pss", bufs=2, space="PSUM") as pss,
        tc.tile_pool(name="ktmp", bufs=3) as ktmp,
    ):
        for ns in range(NP // 512):
            c_t = [
                cs_pool.tile([128, 512], F16, tag=f"c{ct}", name=f"c{ct}")
                for ct in range(NCT)
            ]
            for ct in range(NCT):
                nc.sync.dma_start(
                    out=c_t[ct][:], in_=io["ctxT"][ts(ct, 128), ts(ns, 512)]
                )
            for ot in range(NCT):
                ps = ps2.tile([128, 512], F32, tag="ps", name="ps")
                for ct in range(NCT):
                    nc.tensor.matmul(
                        ps[:],
                        wk_sb[ct][:, ts(ot, 128)],
                        c_t[ct][:],
                        start=(ct == 0),
                        stop=(ct == NCT - 1),
                    )
                kn = kTr[ot][:, ts(ns, 512)]
                nc.vector.tensor_scalar_add(kn, ps[:], bk_sb[:, ot : ot + 1])
                sw = pss.tile([128, 512], F32, tag="sw", name="sw")
                nc.tensor.matmul(sw[:], pswapT[:], kn, start=True, stop=True)
                tmp = ktmp.tile([128, 512], F16, tag="tmp", name="tmp")
                nc.vector.tensor_mul(tmp[:], sw[:], sin2[:, ts(ns, 512)])
                nc.vector.tensor_mul(kn, kn, cos2[:, ts(ns, 512)])
                nc.vector.tensor_add(kn, kn, tmp[:])


def _phase3_vproj(tc, nc, io, v_sb, wv_sb, bv_rep):
    with (
        tc.tile_pool(name="cv", bufs=2) as cv_pool,
        tc.tile_pool(name="ps3", bufs=4, space="PSUM") as ps3,
    ):
        bb = bv_rep.rearrange("p (h d) -> p h d", d=D)
        for kc in range(NP // 128):
            cv_t = [
                cv_pool.tile([128, 128], F16, tag=f"cv{ct}", name=f"cv{ct}")
                for ct in range(NCT)
            ]
            for ct in range(NCT):
                nc.sync.dma_start(
                    out=cv_t[ct][:], in_=io["ctxT"][ts(ct, 128), ts(kc, 128)]
                )
            vv = v_sb[kc].rearrange("p (h c) -> p h c", c=D + 1)
            for oh in range(2):
                ps = ps3.tile([128, 512], F32, tag="ps", name="ps")
                for ct in range(NCT):
                    nc.tensor.matmul(
                        ps[:],
                        cv_t[ct][:],
                        wv_sb[ct][:, ts(oh, 512)],
                        start=(ct == 0),
                        stop=(ct == NCT - 1),
                    )
                nc.vector.tensor_add(
                    vv[:, oh * 8 : (oh + 1) * 8, 0:D],
                    ps.rearrange("p (h d) -> p h d", d=D),
                    bb[:, oh * 8 : (oh + 1) * 8, :],
                )
            nc.vector.memset(vv[:, :, D : D + 1], 1.0)


def _phase4_attention(tc, nc, qT, kTr, v_sb, o_sb, Exp):
    with (
        tc.tile_pool(name="sps", bufs=2, space="PSUM") as sps_pool,
        tc.tile_pool(name="acc", bufs=1, space="PSUM") as acc_pool,
        tc.tile_pool(name="expp", bufs=3) as exp_pool,
        tc.tile_pool(name="dr", bufs=4) as dr_pool,
    ):
        NKC = NP // 128  # 16
        for qh in range(NQ // 512):
            for g in range(H // 2):
                accA = acc_pool.tile([128, 4 * (D + 1)], F32, tag="accA", name="accA")
                accB = acc_pool.tile([128, 4 * (D + 1)], F32, tag="accB", name="accB")
                for ki in range(NKC):
                    sp = sps_pool.tile([128, 1024], F32, tag="sp", name="sp")
                    nc.tensor.matmul(
                        sp[:, 0:512],
                        kTr[g][0:64, ts(ki, 128)],
                        qT[g][0:64, ts(qh, 512)],
                        start=True,
                        stop=True,
                    )
                    nc.tensor.matmul(
                        sp[:, 512:1024],
                        kTr[g][64:128, ts(ki, 128)],
                        qT[g][64:128, ts(qh, 512)],
                        start=True,
                        stop=True,
                    )
                    ex = exp_pool.tile([128, 1024], BF16, tag="ex", name="ex")
                    nc.scalar.activation(ex[:], sp[:], Exp, scale=float(SCALE))
                    for qc in range(4):
                        first = ki == 0 and qc == 0
                        last = ki == NKC - 1 and qc == 3
                        nc.tensor.matmul(
                            accA[:, qc * 65 : qc * 65 + 65],
                            ex[:, ts(qc, 128)],
                            v_sb[ki][:, (2 * g) * 65 : (2 * g) * 65 + 65],
                            start=first,
                            stop=last,
                        )
                        nc.tensor.matmul(
                            accB[:, qc * 65 : qc * 65 + 65],
                            ex[:, 512 + qc * 128 : 512 + qc * 128 + 128],
                            v_sb[ki][:, (2 * g + 1) * 65 : (2 * g + 1) * 65 + 65],
                            start=first,
                            stop=last,
                        )
                for j, acc in ((0, accA), (1, accB)):
                    h = 2 * g + j
                    for qc in range(4):
                        rc = dr_pool.tile([128, 1], F32, tag="rc", name="rc")
                        nc.vector.reciprocal(rc[:], acc[:, qc * 65 + 64 : qc * 65 + 65])
                        nc.vector.tensor_scalar_mul(
                            o_sb[qh * 4 + qc][:, h * D : (h + 1) * D],
                            acc[:, qc * 65 : qc * 65 + 64],
                            rc[:],
                        )


def _phase5_outproj(tc, nc, io, o_sb, wo_sb, bo_rep, ident):
    with (
        tc.tile_pool(name="tp", bufs=4, space="PSUM") as tp_pool,
        tc.tile_pool(name="oT", bufs=2) as oT_pool,
        tc.tile_pool(name="ps5", bufs=4, space="PSUM") as ps5,
        tc.tile_pool(name="ysb", bufs=2) as y_pool,
    ):
        for sc in range(NQ // 128):
            oT_t = []
            for ct in range(NCT):
                tp = tp_pool.tile([128, 128], F16, tag="tp", name="tp")
                nc.tensor.transpose(tp[:], o_sb[sc][:, ts(ct, 128)], ident[:])
                o_t = oT_pool.tile([128, 128], F16, tag=f"oT{ct}", name=f"oT{ct}")
                nc.vector.tensor_copy(o_t[:], tp[:])
                oT_t.append(o_t)
            ysb = y_pool.tile([128, C], F32, tag="ysb", name="ysb")
            for oh in range(2):
                ps = ps5.tile([128, 512], F32, tag="ps", name="ps")
                for ct in range(NCT):
                    nc.tensor.matmul(
                        ps[:],
                        oT_t[ct][:],
                        wo_sb[ct][:, ts(oh, 512)],
                        start=(ct == 0),
                        stop=(ct == NCT - 1),
                    )
                nc.vector.tensor_add(ysb[:, ts(oh, 512)], ps[:], bo_rep[:, ts(oh, 512)])
            nc.sync.dma_start(out=io["y"][ts(sc, 128), :], in_=ysb[:])


# ======================================================================
# Self-contained entry point: kernel(**inputs) with FULL unsharded inputs.
# Shards batch across 8 NeuronCores (data parallel), runs the Bass kernel
# via run_bass_kernel_spmd, returns the FULL [8, 1024, 1024] output.
# ======================================================================

_NC_CACHE = {}


def _build_nc():
    if "nc" in _NC_CACHE:
        return _NC_CACHE["nc"]
    from contextlib import ExitStack
    import concourse.tile as tile
    from concourse import bacc

    nc = bacc.Bacc("TRN2", target_bir_lowering=False, debug=False, num_devices=B)
    io = declare_io(nc)
    with tile.TileContext(nc, trace_sim=False) as tc:
        with ExitStack() as st:
            emit(st, tc, io)
    nc.compile()
    _NC_CACHE["nc"] = nc
    return nc


def kernel(x, ctx, freqs_cis, wq, bq, wk, bk, wv, bv, wo, bo):
    from concourse import bass_utils

    nc = _build_nc()
    in_maps = host_prep(x, ctx, freqs_cis, wq, bq, wk, bk, wv, bv, wo, bo)
    res = bass_utils.run_bass_kernel_spmd(
        nc, in_maps, core_ids=list(range(len(in_maps))), trace=False
    )
    return np.stack([res.results[b]["y"] for b in range(len(in_maps))]).astype(
        np.float32
    )


# revision 11
# speedup vs baseline: 1.2215x; 1.2215x over previous
"""Cross-attention + RoPE Bass/Tile kernel for TRN2 (v3: software-pipelined).

Per-core computation (batch element b = core id). Natural head-dim layout.
All matmuls contract over the partition dim:

  qT  [C, NQ] tiles: qT[t] rows = head-dims of heads 2t, 2t+1 (d natural)
  kTr [C, NP] tiles: same layout, RoPE applied
  RoPE: krot = kn * cos2 + (P_swap @ kn) * sin2  (P_swap = signed pair swap
        done as a PE matmul; cos2/sin2 are pair-duplicated rows)
  scores per head: one matmul, 64-part contraction:
        s[128k, 512q] = kTr[g][64h', ki*128:+128].T @ qT[g][64h', qh*512:+512]
  exp in bf16 (scores/8 reaches ~16.4 > fp16 exp overflow; bf16 is safe)
  PV transposed: acc[128q, 65] += ex[128k, 128q].T @ v_aug[128k, 65]
        (v augmented with a ones column per head -> softmax denominator)
  drain: o_sb[q, c] = acc[:, 0:64] * (1/acc[:, 64])
  out-proj: PE-transpose o_sb -> oT [c, q] tiles, y = oT.T @ woT + bo

Software pipeline: the attention phase is Act-engine-bound (~267us of exp),
so projection work is interleaved into it to keep PE busy during act waits:
  prologue:      Q proj (all tiles) + K tile 0 (+RoPE)
  group (0,0):   V chunks emitted just-in-time before the PV that needs them
  group (0,g):   K tile g+1 (+RoPE) interleaved at 4 points in the ki loop
  group (1,g):   out-projection of query block g (for g<4) interleaved
  tail:          out-projection of query blocks 4-7
All projection/transpose/out-proj PSUM goes through one 2-bank scratch pool
(pp); scores use a 2x2-bank pool; PV accumulators 2x1 bank: 8 banks total.
DMAs are consolidated (one per matrix) to amortize per-DMA HWDGE overhead.
"""

import sys as _s
if "/opt/trn_rl_repo" not in _s.path:
    _s.path.insert(0, "/opt/trn_rl_repo")

import numpy as np

import concourse.bass as bass
import concourse.mybir as mybir
from concourse.bass import ts

F32 = mybir.dt.float32
F16 = mybir.dt.float16
BF16 = mybir.dt.bfloat16

B, NQ, NP, C, H = 8, 1024, 2048, 1024, 16
D = C // H  # 64
NCT = C // 128  # 8 c-tiles
SCALE = 1.0 / np.sqrt(D)
ALL_PHASES = (1, 2, 3, 4, 5)


def host_prep(x, ctx, freqs_cis, wq, bq, wk, bk, wv, bv, wo, bo):
    """Numpy-side layout prep. Returns per-core list of input dicts."""
    f32, f16 = np.float32, np.float16

    cosT = np.ascontiguousarray(np.asarray(freqs_cis, f32)[:, :, 0].T)  # [32, NP]
    sinT = np.ascontiguousarray(np.asarray(freqs_cis, f32)[:, :, 1].T)
    cos2 = np.tile(np.repeat(cosT, 2, axis=0), (2, 1)).astype(f16)
    sin2 = np.tile(np.repeat(sinT, 2, axis=0), (2, 1)).astype(f16)

    # P_swap (as lhsT): out[2p] = -kn[2p+1]; out[2p+1] = +kn[2p]
    pswapT = np.zeros((128, 128), dtype=f16)
    for p in range(64):
        pswapT[2 * p + 1, 2 * p] = -1.0
        pswapT[2 * p, 2 * p + 1] = 1.0
    ident = np.eye(128, dtype=f16)

    shared = {
        "wqT": np.ascontiguousarray(np.asarray(wq, f32).T.astype(f16)),
        "wkT": np.ascontiguousarray(np.asarray(wk, f32).T.astype(f16)),
        "wvT": np.ascontiguousarray(np.asarray(wv, f32).T.astype(f16)),
        "woT": np.ascontiguousarray(np.asarray(wo, f32).T.astype(f16)),
        "bq": np.asarray(bq, f32).copy(),
        "bk": np.asarray(bk, f32).copy(),
        "bv": np.asarray(bv, f32).copy(),
        "bo": np.asarray(bo, f32).copy(),
        "cos2": np.ascontiguousarray(cos2),
        "sin2": np.ascontiguousarray(sin2),
        "pswapT": pswapT,
        "ident": ident,
    }
    per_core = []
    for b in range(x.shape[0]):
        per_core.append({
            "xT": np.ascontiguousarray(np.asarray(x[b], f32).T.astype(f16)),
            "ctxT": np.ascontiguousarray(np.asarray(ctx[b], f32).T.astype(f16)),
            **shared,
        })
    return per_core


def declare_io(nc):
    d = {}
    d["xT"] = nc.dram_tensor("xT", [C, NQ], F16, kind="ExternalInput").ap()
    d["ctxT"] = nc.dram_tensor("ctxT", [C, NP], F16, kind="ExternalInput").ap()
    for w in ("wqT", "wkT", "wvT", "woT"):
        d[w] = nc.dram_tensor(w, [C, C], F16, kind="ExternalInput").ap()
    for bname in ("bq", "bk", "bv", "bo"):
        d[bname] = nc.dram_tensor(bname, [C], F32, kind="ExternalInput").ap()
    d["cos2"] = nc.dram_tensor("cos2", [128, NP], F16, kind="ExternalInput").ap()
    d["sin2"] = nc.dram_tensor("sin2", [128, NP], F16, kind="ExternalInput").ap()
    d["pswapT"] = nc.dram_tensor("pswapT", [128, 128], F16, kind="ExternalInput").ap()
    d["ident"] = nc.dram_tensor("ident", [128, 128], F16, kind="ExternalInput").ap()
    d["y"] = nc.dram_tensor("y", [NQ, C], F16, kind="ExternalOutput").ap()
    return d


def _load_wide(nc, pool, io_ap, name, inner):
    """One consolidated DMA: DRAM [(t p), inner] -> SBUF [128, 8*inner]."""
    t = pool.tile([128, NCT * inner], F16, tag=name, name=name)
    nc.sync.dma_start(
        out=t.rearrange("p (t n) -> p t n", t=NCT),
        in_=io_ap.rearrange("(t p) n -> p t n", p=128),
    )
    return t


class _Ctx:
    """Bag of tiles/pools shared by the emission helpers."""
    pass


def emit(ctx, tc, io, phases=ALL_PHASES):
    """Emit the kernel under an open TileContext. ctx is an ExitStack."""
    nc = tc.nc
    c = _Ctx()
    c.nc = nc
    c.io = io
    c.Exp = mybir.ActivationFunctionType.Exp

    consts = ctx.enter_context(tc.tile_pool(name="consts", bufs=1))
    persist = ctx.enter_context(tc.tile_pool(name="persist", bufs=1))

    # --- persistent activations -----------------------------------------
    c.qT = [persist.tile([128, NQ], F16, tag=f"qT{t}", name=f"qT{t}") for t in range(NCT)]
    c.kTr = [persist.tile([128, NP], F16, tag=f"kTr{t}", name=f"kTr{t}") for t in range(NCT)]
    c.v_sb = [
        persist.tile([128, H * (D + 1)], BF16, tag=f"v{kc}", name=f"v{kc}")
        for kc in range(NP // 128)
    ]

    # --- constants -------------------------------------------------------
    c.bq_sb = consts.tile([128, NCT], F32, tag="bq", name="bq")
    c.bk_sb = consts.tile([128, NCT], F32, tag="bk", name="bk")
    nc.sync.dma_start(out=c.bq_sb[:], in_=io["bq"].rearrange("(t p) -> p t", p=128))
    nc.sync.dma_start(out=c.bk_sb[:], in_=io["bk"].rearrange("(t p) -> p t", p=128))
    c.bv_rep = consts.tile([128, C], F32, tag="bv_rep", name="bv_rep")
    c.bo_rep = consts.tile([128, C], F32, tag="bo_rep", name="bo_rep")
    c.ident = consts.tile([128, 128], F16, tag="ident", name="ident")
    nc.sync.dma_start(out=c.ident[:], in_=io["ident"][:])
    with tc.tile_pool(name="rows", bufs=1) as rows:
        bv_row = rows.tile([1, C], F32, tag="bv_row", name="bv_row")
        bo_row = rows.tile([1, C], F32, tag="bo_row", name="bo_row")
        nc.sync.dma_start(out=bv_row[:], in_=io["bv"].unsqueeze(0))
        nc.sync.dma_start(out=bo_row[:], in_=io["bo"].unsqueeze(0))
        nc.gpsimd.partition_broadcast(c.bv_rep[:], bv_row[:], channels=128)
        nc.gpsimd.partition_broadcast(c.bo_rep[:], bo_row[:], channels=128)

    # --- weight/input pools ----------------------------------------------
    kq = ctx.enter_context(tc.tile_pool(name="kq", bufs=1))
    ctxp = ctx.enter_context(tc.tile_pool(name="ctxp", bufs=1))
    # shared psum scratch for all projection / transpose / out-proj matmuls
    c.pp = ctx.enter_context(tc.tile_pool(name="pp", bufs=2, space="PSUM"))
    c.ktmp_pool = ctx.enter_context(tc.tile_pool(name="ktmp", bufs=2))

    with tc.tile_pool(name="qx", bufs=1) as qx:
        # DMA order = need order: wq+x (Q proj), wk+ctx (K), wv (V)
        c.wq_w = _load_wide(nc, qx, io["wqT"], "wq", C)
        c.x_w = _load_wide(nc, qx, io["xT"], "xw", NQ)
        c.wk_w = _load_wide(nc, kq, io["wkT"], "wk", C)
        c.ctx_w = _load_wide(nc, ctxp, io["ctxT"], "ctxw", NP)
        c.wv_w = _load_wide(nc, ctxp, io["wvT"], "wv", C)
        c.cos2 = kq.tile([128, NP], F16, tag="cos2", name="cos2")
        c.sin2 = kq.tile([128, NP], F16, tag="sin2", name="sin2")
        nc.sync.dma_start(out=c.cos2[:], in_=io["cos2"][:])
        nc.sync.dma_start(out=c.sin2[:], in_=io["sin2"][:])
        c.pswapT = kq.tile([128, 128], F16, tag="pswapT", name="pswapT")
        nc.sync.dma_start(out=c.pswapT[:], in_=io["pswapT"][:])

        # --- prologue: all of Q, K tile 0 --------------------------------
        _q_tile(c, 0)
        for ns in range(NP // 512):
            _k_block(c, 0, ns)
        for ot in range(1, NCT):
            _q_tile(c, ot)

    # --- attention + interleaved K tiles / V chunks / out-proj -----------
    with (
        tc.tile_pool(name="wop", bufs=1) as wop,
        tc.tile_pool(name="sps", bufs=2, space="PSUM") as sps_pool,
        tc.tile_pool(name="acc", bufs=1, space="PSUM") as acc_pool,
        tc.tile_pool(name="expp", bufs=3) as exp_pool,
        tc.tile_pool(name="dr", bufs=4) as dr_pool,
        tc.tile_pool(name="oT", bufs=1) as oT_pool,
        tc.tile_pool(name="ysb", bufs=2) as y_pool,
    ):
        c.wo_w = _load_wide(nc, wop, io["woT"], "wo", C)
        c.o_sb = [
            wop.tile([128, C], F16, tag=f"o{sc}", name=f"o{sc}")
            for sc in range(NQ // 128)
        ]
        c.sps_pool, c.acc_pool = sps_pool, acc_pool
        c.exp_pool, c.dr_pool = exp_pool, dr_pool
        c.oT_pool, c.y_pool = oT_pool, y_pool

        NKC = NP // 128  # 16
        for qh in range(2):
            for g in range(8):
                inject = {}
                if qh == 0:
                    if g == 0:
                        # V chunks just-in-time: v_sb[ki] right before PV ki
                        for ki in range(NKC):
                            inject.setdefault(ki, []).append(
                                (lambda kk: lambda: _v_chunk(c, kk))(ki)
                            )
                    if g < 7:
                        # K tile g+1: its 4 ns-blocks at ki 1, 5, 9, 13
                        for bi, ki in enumerate((1, 5, 9, 13)):
                            inject.setdefault(ki, []).append(
                                (lambda o, n: lambda: _k_block(c, o, n))(g + 1, bi)
                            )
                else:
                    if g < 4:
                        inject.setdefault(2, []).append(
                            (lambda s: lambda: _outproj_sc(c, s))(g)
                        )
                _att_group(c, qh, g, inject)
        for sc in range(4, 8):
            _outproj_sc(c, sc)


def _q_tile(c, ot):
    nc = c.nc
    for qs in range(NQ // 512):
        ps = c.pp.tile([128, 512], F32, tag="pp", name="pp")
        for ct in range(NCT):
            nc.tensor.matmul(
                ps[:],
                c.wq_w[:, ct * C + ot * 128 : ct * C + ot * 128 + 128],
                c.x_w[:, ct * NQ + qs * 512 : ct * NQ + qs * 512 + 512],
                start=(ct == 0),
                stop=(ct == NCT - 1),
            )
        nc.vector.tensor_scalar_add(
            c.qT[ot][:, ts(qs, 512)], ps[:], c.bq_sb[:, ot : ot + 1]
        )


def _k_block(c, ot, ns):
    """K projection + RoPE for tile ot, one 512-column n-block."""
    nc = c.nc
    ps = c.pp.tile([128, 512], F32, tag="pp", name="pp")
    for ct in range(NCT):
        nc.tensor.matmul(
            ps[:],
            c.wk_w[:, ct * C + ot * 128 : ct * C + ot * 128 + 128],
            c.ctx_w[:, ct * NP + ns * 512 : ct * NP + ns * 512 + 512],
            start=(ct == 0),
            stop=(ct == NCT - 1),
        )
    kn = c.kTr[ot][:, ts(ns, 512)]
    nc.vector.tensor_scalar_add(kn, ps[:], c.bk_sb[:, ot : ot + 1])
    sw = c.pp.tile([128, 512], F32, tag="pp", name="pp")
    nc.tensor.matmul(sw[:], c.pswapT[:], kn, start=True, stop=True)
    tmp = c.ktmp_pool.tile([128, 512], F16, tag="ktmp", name="ktmp")
    nc.vector.tensor_mul(tmp[:], sw[:], c.sin2[:, ts(ns, 512)])
    nc.vector.tensor_mul(kn, kn, c.cos2[:, ts(ns, 512)])
    nc.vector.tensor_add(kn, kn, tmp[:])


def _v_chunk(c, kc):
    """V projection for one 128-key chunk (+bias, ones column)."""
    nc = c.nc
    vv = c.v_sb[kc].rearrange("p (h c) -> p h c", c=D + 1)
    bb = c.bv_rep.rearrange("p (h d) -> p h d", d=D)
    for oh in range(2):
        ps = c.pp.tile([128, 512], F32, tag="pp", name="pp")
        for ct in range(NCT):
            nc.tensor.matmul(
                ps[:],
                c.ctx_w[:, ct * NP + kc * 128 : ct * NP + kc * 128 + 128],
                c.wv_w[:, ct * C + oh * 512 : ct * C + oh * 512 + 512],
                start=(ct == 0),
                stop=(ct == NCT - 1),
            )
        nc.vector.tensor_add(
            vv[:, oh * 8 : (oh + 1) * 8, 0:D],
            ps.rearrange("p (h d) -> p h d", d=D),
            bb[:, oh * 8 : (oh + 1) * 8, :],
        )
    nc.vector.memset(vv[:, :, D : D + 1], 1.0)


def _att_group(c, qh, g, inject):
    """One attention group: head pair (2g, 2g+1) x 512 queries."""
    nc = c.nc
    NKC = NP // 128
    accA = c.acc_pool.tile([128, 4 * (D + 1)], F32, tag="accA", name="accA")
    accB = c.acc_pool.tile([128, 4 * (D + 1)], F32, tag="accB", name="accB")
    for ki in range(NKC):
        sp = c.sps_pool.tile([128, 1024], F32, tag="sp", name="sp")
        nc.tensor.matmul(
            sp[:, 0:512],
            c.kTr[g][0:64, ts(ki, 128)],
            c.qT[g][0:64, ts(qh, 512)],
            start=True,
            stop=True,
        )
        nc.tensor.matmul(
            sp[:, 512:1024],
            c.kTr[g][64:128, ts(ki, 128)],
            c.qT[g][64:128, ts(qh, 512)],
            start=True,
            stop=True,
        )
        ex = c.exp_pool.tile([128, 1024], BF16, tag="ex", name="ex")
        nc.scalar.activation(ex[:], sp[:], c.Exp, scale=float(SCALE))
        for fn in inject.get(ki, ()):
            fn()
        for qc in range(4):
            first = ki == 0 and qc == 0
            last = ki == NKC - 1 and qc == 3
            nc.tensor.matmul(
                accA[:, qc * 65 : qc * 65 + 65],
                ex[:, ts(qc, 128)],
                c.v_sb[ki][:, (2 * g) * 65 : (2 * g) * 65 + 65],
                start=first,
                stop=last,
            )
            nc.tensor.matmul(
                accB[:, qc * 65 : qc * 65 + 65],
                ex[:, 512 + qc * 128 : 512 + qc * 128 + 128],
                c.v_sb[ki][:, (2 * g + 1) * 65 : (2 * g + 1) * 65 + 65],
                start=first,
                stop=last,
            )
    for j, acc in ((0, accA), (1, accB)):
        h = 2 * g + j
        for qc in range(4):
            rc = c.dr_pool.tile([128, 1], F32, tag="rc", name="rc")
            nc.vector.reciprocal(rc[:], acc[:, qc * 65 + 64 : qc * 65 + 65])
            nc.vector.tensor_scalar_mul(
                c.o_sb[qh * 4 + qc][:, h * D : (h + 1) * D],
                acc[:, qc * 65 : qc * 65 + 64],
                rc[:],
            )


def _outproj_sc(c, sc):
    """Out-projection for one 128-query block: transpose + matmul + store."""
    nc = c.nc
    oT_t = []
    for ct in range(NCT):
        tp = c.pp.tile([128, 128], F16, tag="pp", name="tp")
        nc.tensor.transpose(tp[:], c.o_sb[sc][:, ts(ct, 128)], c.ident[:])
        o_t = c.oT_pool.tile([128, 128], F16, tag=f"oT{ct}", name=f"oT{ct}")
        nc.vector.tensor_copy(o_t[:], tp[:])
        oT_t.append(o_t)
    for oh in range(2):
        ps = c.pp.tile([128, 512], F32, tag="pp", name="ppy")
        for ct in range(NCT):
            nc.tensor.matmul(
                ps[:],
                oT_t[ct][:],
                c.wo_w[:, ct * C + oh * 512 : ct * C + oh * 512 + 512],
                start=(ct == 0),
                stop=(ct == NCT - 1),
            )
        ysb = c.y_pool.tile([128, 512], F16, tag="ysb", name="ysb")
        nc.vector.tensor_add(ysb[:], ps[:], c.bo_rep[:, ts(oh, 512)])
        nc.sync.dma_start(out=c.io["y"][ts(sc, 128), ts(oh, 512)], in_=ysb[:])


# ======================================================================
# Self-contained entry point: kernel(**inputs) with FULL unsharded inputs.
# Shards batch across 8 NeuronCores (data parallel), runs the Bass kernel
# via run_bass_kernel_spmd, returns the FULL [8, 1024, 1024] output.
# ======================================================================

_NC_CACHE = {}


def _build_nc():
    if "nc" in _NC_CACHE:
        return _NC_CACHE["nc"]
    from contextlib import ExitStack
    import concourse.tile as tile
    from concourse import bacc

    nc = bacc.Bacc("TRN2", target_bir_lowering=False, debug=False, num_devices=B)
    io = declare_io(nc)
    with tile.TileContext(nc, trace_sim=False) as tc:
        with ExitStack() as st:
            emit(st, tc, io)
    nc.compile()
    _NC_CACHE["nc"] = nc
    return nc


def kernel(x, ctx, freqs_cis, wq, bq, wk, bk, wv, bv, wo, bo):
    from concourse import bass_utils

    nc = _build_nc()
    in_maps = host_prep(x, ctx, freqs_cis, wq, bq, wk, bk, wv, bv, wo, bo)
    res = bass_utils.run_bass_kernel_spmd(
        nc, in_maps, core_ids=list(range(len(in_maps))), trace=False
    )
    return np.stack([res.results[b]["y"] for b in range(len(in_maps))]).astype(
        np.float32
    )


# revision 12
# speedup vs baseline: 1.2479x; 1.0216x over previous
"""Cross-attention + RoPE Bass/Tile kernel for TRN2 (v3: software-pipelined).

Per-core computation (batch element b = core id). Natural head-dim layout.
All matmuls contract over the partition dim:

  qT  [C, NQ] tiles: qT[t] rows = head-dims of heads 2t, 2t+1 (d natural)
  kTr [C, NP] tiles: same layout, RoPE applied
  RoPE: krot = kn * cos2 + (P_swap @ kn) * sin2  (P_swap = signed pair swap
        done as a PE matmul; cos2/sin2 are pair-duplicated rows)
  scores per head: one matmul, 64-part contraction:
        s[128k, 512q] = kTr[g][64h', ki*128:+128].T @ qT[g][64h', qh*512:+512]
  exp in bf16 (scores/8 reaches ~16.4 > fp16 exp overflow; bf16 is safe)
  PV transposed: acc[128q, 65] += ex[128k, 128q].T @ v_aug[128k, 65]
        (v augmented with a ones column per head -> softmax denominator)
  drain: o_sb[q, c] = acc[:, 0:64] * (1/acc[:, 64])
  out-proj: PE-transpose o_sb -> oT [c, q] tiles, y = oT.T @ woT + bo

Software pipeline: the attention phase is Act-engine-bound (~267us of exp),
so projection work is interleaved into it to keep PE busy during act waits:
  prologue:      Q proj (all tiles) + K tile 0 (+RoPE)
  group (0,0):   V chunks emitted just-in-time before the PV that needs them
  group (0,g):   K tile g+1 (+RoPE) interleaved at 4 points in the ki loop
  group (1,g):   out-projection of query block g (for g<4) interleaved
  tail:          out-projection of query blocks 4-7
All projection/transpose/out-proj PSUM goes through one 2-bank scratch pool
(pp); scores use a 2x2-bank pool; PV accumulators 2x1 bank: 8 banks total.
DMAs are consolidated (one per matrix) to amortize per-DMA HWDGE overhead.
"""

import sys as _s
if "/opt/trn_rl_repo" not in _s.path:
    _s.path.insert(0, "/opt/trn_rl_repo")

import numpy as np

import concourse.bass as bass
import concourse.mybir as mybir
from concourse.bass import ts

F32 = mybir.dt.float32
F16 = mybir.dt.float16
BF16 = mybir.dt.bfloat16

B, NQ, NP, C, H = 8, 1024, 2048, 1024, 16
D = C // H  # 64
NCT = C // 128  # 8 c-tiles
SCALE = 1.0 / np.sqrt(D)
ALL_PHASES = (1, 2, 3, 4, 5)


def host_prep(x, ctx, freqs_cis, wq, bq, wk, bk, wv, bv, wo, bo):
    """Numpy-side layout prep. Returns per-core list of input dicts."""
    f32, f16 = np.float32, np.float16

    cosT = np.ascontiguousarray(np.asarray(freqs_cis, f32)[:, :, 0].T)  # [32, NP]
    sinT = np.ascontiguousarray(np.asarray(freqs_cis, f32)[:, :, 1].T)
    cos2 = np.tile(np.repeat(cosT, 2, axis=0), (2, 1)).astype(f16)
    sin2 = np.tile(np.repeat(sinT, 2, axis=0), (2, 1)).astype(f16)

    # P_swap (as lhsT): out[2p] = -kn[2p+1]; out[2p+1] = +kn[2p]
    pswapT = np.zeros((128, 128), dtype=f16)
    for p in range(64):
        pswapT[2 * p + 1, 2 * p] = -1.0
        pswapT[2 * p, 2 * p + 1] = 1.0
    ident = np.eye(128, dtype=f16)

    shared = {
        "wqT": np.ascontiguousarray(np.asarray(wq, f32).T.astype(f16)),
        "wkT": np.ascontiguousarray(np.asarray(wk, f32).T.astype(f16)),
        "wvT": np.ascontiguousarray(np.asarray(wv, f32).T.astype(f16)),
        "woT": np.ascontiguousarray(np.asarray(wo, f32).T.astype(f16)),
        "bq": np.asarray(bq, f32).copy(),
        "bk": np.asarray(bk, f32).copy(),
        "bv": np.asarray(bv, f32).copy(),
        "bo": np.asarray(bo, f32).copy(),
        "cos2": np.ascontiguousarray(cos2),
        "sin2": np.ascontiguousarray(sin2),
        "pswapT": pswapT,
        "ident": ident,
    }
    per_core = []
    for b in range(x.shape[0]):
        per_core.append({
            "xT": np.ascontiguousarray(np.asarray(x[b], f32).T.astype(f16)),
            "ctxT": np.ascontiguousarray(np.asarray(ctx[b], f32).T.astype(f16)),
            **shared,
        })
    return per_core


def declare_io(nc):
    d = {}
    d["xT"] = nc.dram_tensor("xT", [C, NQ], F16, kind="ExternalInput").ap()
    d["ctxT"] = nc.dram_tensor("ctxT", [C, NP], F16, kind="ExternalInput").ap()
    for w in ("wqT", "wkT", "wvT", "woT"):
        d[w] = nc.dram_tensor(w, [C, C], F16, kind="ExternalInput").ap()
    for bname in ("bq", "bk", "bv", "bo"):
        d[bname] = nc.dram_tensor(bname, [C], F32, kind="ExternalInput").ap()
    d["cos2"] = nc.dram_tensor("cos2", [128, NP], F16, kind="ExternalInput").ap()
    d["sin2"] = nc.dram_tensor("sin2", [128, NP], F16, kind="ExternalInput").ap()
    d["pswapT"] = nc.dram_tensor("pswapT", [128, 128], F16, kind="ExternalInput").ap()
    d["ident"] = nc.dram_tensor("ident", [128, 128], F16, kind="ExternalInput").ap()
    d["y"] = nc.dram_tensor("y", [NQ, C], F16, kind="ExternalOutput").ap()
    return d


def _load_wide(nc, pool, io_ap, name, inner):
    """One consolidated DMA: DRAM [(t p), inner] -> SBUF [128, 8*inner]."""
    t = pool.tile([128, NCT * inner], F16, tag=name, name=name)
    nc.sync.dma_start(
        out=t.rearrange("p (t n) -> p t n", t=NCT),
        in_=io_ap.rearrange("(t p) n -> p t n", p=128),
    )
    return t


class _Ctx:
    """Bag of tiles/pools shared by the emission helpers."""
    pass


def emit(ctx, tc, io, phases=ALL_PHASES):
    """Emit the kernel under an open TileContext. ctx is an ExitStack."""
    nc = tc.nc
    c = _Ctx()
    c.nc = nc
    c.io = io
    c.Exp = mybir.ActivationFunctionType.Exp

    consts = ctx.enter_context(tc.tile_pool(name="consts", bufs=1))
    persist = ctx.enter_context(tc.tile_pool(name="persist", bufs=1))

    # --- persistent activations -----------------------------------------
    c.qT = [persist.tile([128, NQ], F16, tag=f"qT{t}", name=f"qT{t}") for t in range(NCT)]
    c.kTr = [persist.tile([128, NP], F16, tag=f"kTr{t}", name=f"kTr{t}") for t in range(NCT)]
    c.v_sb = [
        persist.tile([128, H * (D + 1)], BF16, tag=f"v{kc}", name=f"v{kc}")
        for kc in range(NP // 128)
    ]

    # --- constants -------------------------------------------------------
    c.bq_sb = consts.tile([128, NCT], F32, tag="bq", name="bq")
    c.bk_sb = consts.tile([128, NCT], F32, tag="bk", name="bk")
    nc.sync.dma_start(out=c.bq_sb[:], in_=io["bq"].rearrange("(t p) -> p t", p=128))
    nc.sync.dma_start(out=c.bk_sb[:], in_=io["bk"].rearrange("(t p) -> p t", p=128))
    c.bv_rep = consts.tile([128, C], F32, tag="bv_rep", name="bv_rep")
    c.bo_rep = consts.tile([128, C], F32, tag="bo_rep", name="bo_rep")
    c.ident = consts.tile([128, 128], F16, tag="ident", name="ident")
    nc.sync.dma_start(out=c.ident[:], in_=io["ident"][:])
    with tc.tile_pool(name="rows", bufs=1) as rows:
        bv_row = rows.tile([1, C], F32, tag="bv_row", name="bv_row")
        bo_row = rows.tile([1, C], F32, tag="bo_row", name="bo_row")
        nc.sync.dma_start(out=bv_row[:], in_=io["bv"].unsqueeze(0))
        nc.sync.dma_start(out=bo_row[:], in_=io["bo"].unsqueeze(0))
        nc.gpsimd.partition_broadcast(c.bv_rep[:], bv_row[:], channels=128)
        nc.gpsimd.partition_broadcast(c.bo_rep[:], bo_row[:], channels=128)

    # --- weight/input pools ----------------------------------------------
    kq = ctx.enter_context(tc.tile_pool(name="kq", bufs=1))
    ctxp = ctx.enter_context(tc.tile_pool(name="ctxp", bufs=1))
    # shared psum scratch for all projection / transpose / out-proj matmuls
    c.pp = ctx.enter_context(tc.tile_pool(name="pp", bufs=2, space="PSUM"))
    c.ktmp_pool = ctx.enter_context(tc.tile_pool(name="ktmp", bufs=2))

    with tc.tile_pool(name="qx", bufs=1) as qx:
        # DMA order = need order: wq+x (Q proj), wk+ctx (K), wv (V)
        c.wq_w = _load_wide(nc, qx, io["wqT"], "wq", C)
        c.x_w = _load_wide(nc, qx, io["xT"], "xw", NQ)
        c.wk_w = _load_wide(nc, kq, io["wkT"], "wk", C)
        c.ctx_w = _load_wide(nc, ctxp, io["ctxT"], "ctxw", NP)
        c.wv_w = _load_wide(nc, ctxp, io["wvT"], "wv", C)
        c.cos2 = kq.tile([128, NP], F16, tag="cos2", name="cos2")
        c.sin2 = kq.tile([128, NP], F16, tag="sin2", name="sin2")
        nc.sync.dma_start(out=c.cos2[:], in_=io["cos2"][:])
        nc.sync.dma_start(out=c.sin2[:], in_=io["sin2"][:])
        c.pswapT = kq.tile([128, 128], F16, tag="pswapT", name="pswapT")
        nc.sync.dma_start(out=c.pswapT[:], in_=io["pswapT"][:])

        # --- prologue: all of Q, K tile 0 --------------------------------
        _q_tile(c, 0)
        for ns in range(NP // 512):
            _k_block(c, 0, ns)
        for ot in range(1, NCT):
            _q_tile(c, ot)

    # --- attention + interleaved K tiles / V chunks / out-proj -----------
    with (
        tc.tile_pool(name="wop", bufs=1) as wop,
        tc.tile_pool(name="sps", bufs=2, space="PSUM") as sps_pool,
        tc.tile_pool(name="acc", bufs=1, space="PSUM") as acc_pool,
        tc.tile_pool(name="expp", bufs=4) as exp_pool,
        tc.tile_pool(name="dr", bufs=4) as dr_pool,
        tc.tile_pool(name="oT", bufs=1) as oT_pool,
        tc.tile_pool(name="ysb", bufs=2) as y_pool,
    ):
        c.wo_w = _load_wide(nc, wop, io["woT"], "wo", C)
        c.o_sb = [
            wop.tile([128, C], F16, tag=f"o{sc}", name=f"o{sc}")
            for sc in range(NQ // 128)
        ]
        c.sps_pool, c.acc_pool = sps_pool, acc_pool
        c.exp_pool, c.dr_pool = exp_pool, dr_pool
        c.oT_pool, c.y_pool = oT_pool, y_pool

        NKC = NP // 128  # 16
        for qh in range(2):
            for g in range(8):
                inject = {}
                if qh == 0:
                    if g == 0:
                        # V chunks just-in-time: v_sb[ki] right before PV ki
                        for ki in range(NKC):
                            inject.setdefault(ki, []).append(
                                (lambda kk: lambda: _v_chunk(c, kk))(ki)
                            )
                    if g < 7:
                        # K tile g+1: its 4 ns-blocks at ki 1, 5, 9, 13
                        for bi, ki in enumerate((1, 5, 9, 13)):
                            inject.setdefault(ki, []).append(
                                (lambda o, n: lambda: _k_block(c, o, n))(g + 1, bi)
                            )
                else:
                    if g < 4:
                        inject.setdefault(2, []).append(
                            (lambda s: lambda: _outproj_sc(c, s))(g)
                        )
                _att_group(c, qh, g, inject)
        for sc in range(4, 8):
            _outproj_sc(c, sc)


def _q_tile(c, ot):
    nc = c.nc
    for qs in range(NQ // 512):
        ps = c.pp.tile([128, 512], F32, tag="pp", name="pp")
        for ct in range(NCT):
            nc.tensor.matmul(
                ps[:],
                c.wq_w[:, ct * C + ot * 128 : ct * C + ot * 128 + 128],
                c.x_w[:, ct * NQ + qs * 512 : ct * NQ + qs * 512 + 512],
                start=(ct == 0),
                stop=(ct == NCT - 1),
            )
        nc.vector.tensor_scalar_add(
            c.qT[ot][:, ts(qs, 512)], ps[:], c.bq_sb[:, ot : ot + 1]
        )


def _k_block(c, ot, ns):
    """K projection + RoPE for tile ot, one 512-column n-block."""
    nc = c.nc
    ps = c.pp.tile([128, 512], F32, tag="pp", name="pp")
    for ct in range(NCT):
        nc.tensor.matmul(
            ps[:],
            c.wk_w[:, ct * C + ot * 128 : ct * C + ot * 128 + 128],
            c.ctx_w[:, ct * NP + ns * 512 : ct * NP + ns * 512 + 512],
            start=(ct == 0),
            stop=(ct == NCT - 1),
        )
    kn = c.kTr[ot][:, ts(ns, 512)]
    nc.vector.tensor_scalar_add(kn, ps[:], c.bk_sb[:, ot : ot + 1])
    sw = c.pp.tile([128, 512], F32, tag="pp", name="pp")
    nc.tensor.matmul(sw[:], c.pswapT[:], kn, start=True, stop=True)
    tmp = c.ktmp_pool.tile([128, 512], F16, tag="ktmp", name="ktmp")
    nc.vector.tensor_mul(tmp[:], sw[:], c.sin2[:, ts(ns, 512)])
    nc.vector.tensor_mul(kn, kn, c.cos2[:, ts(ns, 512)])
    nc.vector.tensor_add(kn, kn, tmp[:])


def _v_chunk(c, kc):
    """V projection for one 128-key chunk (+bias, ones column)."""
    nc = c.nc
    vv = c.v_sb[kc].rearrange("p (h c) -> p h c", c=D + 1)
    bb = c.bv_rep.rearrange("p (h d) -> p h d", d=D)
    for oh in range(2):
        ps = c.pp.tile([128, 512], F32, tag="pp", name="pp")
        for ct in range(NCT):
            nc.tensor.matmul(
                ps[:],
                c.ctx_w[:, ct * NP + kc * 128 : ct * NP + kc * 128 + 128],
                c.wv_w[:, ct * C + oh * 512 : ct * C + oh * 512 + 512],
                start=(ct == 0),
                stop=(ct == NCT - 1),
            )
        nc.vector.tensor_add(
            vv[:, oh * 8 : (oh + 1) * 8, 0:D],
            ps.rearrange("p (h d) -> p h d", d=D),
            bb[:, oh * 8 : (oh + 1) * 8, :],
        )
    nc.vector.memset(vv[:, :, D : D + 1], 1.0)


def _att_group(c, qh, g, inject):
    """One attention group: head pair (2g, 2g+1) x 512 queries."""
    nc = c.nc
    NKC = NP // 128
    accA = c.acc_pool.tile([128, 4 * (D + 1)], F32, tag="accA", name="accA")
    accB = c.acc_pool.tile([128, 4 * (D + 1)], F32, tag="accB", name="accB")
    for ki in range(NKC):
        sp = c.sps_pool.tile([128, 1024], F32, tag="sp", name="sp")
        nc.tensor.matmul(
            sp[:, 0:512],
            c.kTr[g][0:64, ts(ki, 128)],
            c.qT[g][0:64, ts(qh, 512)],
            start=True,
            stop=True,
        )
        nc.tensor.matmul(
            sp[:, 512:1024],
            c.kTr[g][64:128, ts(ki, 128)],
            c.qT[g][64:128, ts(qh, 512)],
            start=True,
            stop=True,
        )
        ex = c.exp_pool.tile([128, 1024], BF16, tag="ex", name="ex")
        nc.scalar.activation(ex[:], sp[:], c.Exp, scale=float(SCALE))
        for fn in inject.get(ki, ()):
            fn()
        for qc in range(4):
            first = ki == 0 and qc == 0
            last = ki == NKC - 1 and qc == 3
            nc.tensor.matmul(
                accA[:, qc * 65 : qc * 65 + 65],
                ex[:, ts(qc, 128)],
                c.v_sb[ki][:, (2 * g) * 65 : (2 * g) * 65 + 65],
                start=first,
                stop=last,
            )
            nc.tensor.matmul(
                accB[:, qc * 65 : qc * 65 + 65],
                ex[:, 512 + qc * 128 : 512 + qc * 128 + 128],
                c.v_sb[ki][:, (2 * g + 1) * 65 : (2 * g + 1) * 65 + 65],
                start=first,
                stop=last,
            )
    for j, acc in ((0, accA), (1, accB)):
        h = 2 * g + j
        for qc in range(4):
            rc = c.dr_pool.tile([128, 1], F32, tag="rc", name="rc")
            nc.vector.reciprocal(rc[:], acc[:, qc * 65 + 64 : qc * 65 + 65])
            nc.vector.tensor_scalar_mul(
                c.o_sb[qh * 4 + qc][:, h * D : (h + 1) * D],
                acc[:, qc * 65 : qc * 65 + 64],
                rc[:],
            )


def _outproj_sc(c, sc):
    """Out-projection for one 128-query block: transpose + matmul + store."""
    nc = c.nc
    oT_t = []
    for ct in range(NCT):
        tp = c.pp.tile([128, 128], F16, tag="pp", name="tp")
        nc.tensor.transpose(tp[:], c.o_sb[sc][:, ts(ct, 128)], c.ident[:])
        o_t = c.oT_pool.tile([128, 128], F16, tag=f"oT{ct}", name=f"oT{ct}")
        nc.vector.tensor_copy(o_t[:], tp[:])
        oT_t.append(o_t)
    for oh in range(2):
        ps = c.pp.tile([128, 512], F32, tag="pp", name="ppy")
        for ct in range(NCT):
            nc.tensor.matmul(
                ps[:],
                oT_t[ct][:],
                c.wo_w[:, ct * C + oh * 512 : ct * C + oh * 512 + 512],
                start=(ct == 0),
                stop=(ct == NCT - 1),
            )
        ysb = c.y_pool.tile([128, 512], F16, tag="ysb", name="ysb")
        nc.vector.tensor_add(ysb[:], ps[:], c.bo_rep[:, ts(oh, 512)])
        nc.sync.dma_start(out=c.io["y"][ts(sc, 128), ts(oh, 512)], in_=ysb[:])


# ======================================================================
# Self-contained entry point: kernel(**inputs) with FULL unsharded inputs.
# Shards batch across 8 NeuronCores (data parallel), runs the Bass kernel
# via run_bass_kernel_spmd, returns the FULL [8, 1024, 1024] output.
# ======================================================================

_NC_CACHE = {}


def _build_nc():
    if "nc" in _NC_CACHE:
        return _NC_CACHE["nc"]
    from contextlib import ExitStack
    import concourse.tile as tile
    from concourse import bacc

    nc = bacc.Bacc("TRN2", target_bir_lowering=False, debug=False, num_devices=B)
    io = declare_io(nc)
    with tile.TileContext(nc, trace_sim=False) as tc:
        with ExitStack() as st:
            emit(st, tc, io)
    nc.compile()
    _NC_CACHE["nc"] = nc
    return nc


def kernel(x, ctx, freqs_cis, wq, bq, wk, bk, wv, bv, wo, bo):
    from concourse import bass_utils

    nc = _build_nc()
    in_maps = host_prep(x, ctx, freqs_cis, wq, bq, wk, bk, wv, bv, wo, bo)
    res = bass_utils.run_bass_kernel_spmd(
        nc, in_maps, core_ids=list(range(len(in_maps))), trace=False
    )
    return np.stack([res.results[b]["y"] for b in range(len(in_maps))]).astype(
        np.float32
    )
